# revision 24
# baseline (speedup 1.0000x reference)
"""CRF loss (log-likelihood sum) on 8 Trainium2 NeuronCores.

Shapes (hardcoded): emissions (512, 512, 128) f32, tags (512, 512) i64,
mask (512, 512) bool (assumed all ones), start/end (128,) f32,
transitions (128, 128) f32.  Output: scalar f32 = sum_b llh_b.

Strategy:
  Numerator (path score) is pure index arithmetic over the inputs and is
  computed on the host in float64.

  Denominator (forward algorithm) in probability space:
      P_t = (E^T @ P_{t-1}) * exp(em_t - g),  E = exp(trans)
  i.e. the per-step logsumexp becomes a TensorE matmul (E stationary)
  followed by one elementwise multiply reading PSUM.  g is a constant
  per-step normalizer chosen so the state stays in bf16 range (validated
  offline); no renormalization needed.

  The recurrence is latency-bound (sequential in t), so the chain is cut
  in half: the identity Z_b = sum_{j,k} P_255[j,b] E[j,k] R'_256[k,b]
  splits the work into a forward recurrence over t=0..255 and an
  independent backward recurrence R'_tau = x_tau * (E @ R'_tau+1) over
  tau=511..256.  Both have the same dataflow (state = x * (W^T @ state)),
  differing only in data: W = exp(trans) vs exp(trans^T), initial bias =
  start vs end, and the order of the emission stream.  Cores 0-3 run
  forward for 128 batch columns each; cores 4-7 run backward for the same
  columns.  Each core runs one SPMD program for 256 sequential steps
  (half of the 511-step chain), and the host joins the two 128x128 final
  state tiles per column block in float64.
"""

import numpy as np

B, T, K = 512, 512, 128
NCORES = 8
BCC = 128                 # batch columns per core (4 fwd + 4 bwd cores)
TCHUNK = 32
NCHUNK = 8                # 8 chunks x 32 = 256 stream positions per core
NSTEP = NCHUNK * TCHUNK   # 256
G = 5.35                  # per-step growth normalizer (exp stays in range)

_PROGRAM = None


def _dedupe_ldweights(nc):
    """Remove redundant weight reloads: every DP-step matmul uses the same
    stationary weights, and the per-matmul LDWEIGHTS sits on the PE queue.
    The tile scheduler splits each matmul into a standalone InstLdweights
    plus a non-self-loading InstMatmult (ldweights=False), so dropping an
    InstLdweights whose weights are already resident is safe."""
    import concourse.mybir as mybir

    def sig(ap):
        try:
            if ap.regs_read():
                return None  # register-offset APs are not statically stable
            return (ap.memref, str(ap.ap), int(ap.offset), str(ap.dtype))
        except Exception:
            return None

    removed = 0
    for blk in nc.main_func.blocks:
        loaded = None
        keep = []
        for inst in blk.instructions:
            if isinstance(inst, mybir.InstLdweights):
                si = inst.sync_info
                clean = si is None or (not si.on_wait and not si.on_update)
                s = sig(inst.ins[0]) if len(inst.ins) == 1 else None
                if s is not None and s == loaded:
                    removed += 1
                    if not clean:
                        # preserve the load's sync as a PE event-sem wait
                        ev = mybir.InstEventSemaphore(
                            name=nc.get_next_instruction_name(), ins=[], outs=[]
                        )
                        ev.engine = mybir.EngineType.PE
                        ev.sync_info = inst.sync_info
                        nc.register_instruction(ev)
                        keep.append(ev)
                    continue  # weights already resident: drop the reload
                loaded = s
            elif (
                isinstance(inst, mybir.InstMatmult)
                and getattr(inst, "is_transpose", False)
            ):
                loaded = None  # transposes stream through the PE array
            keep.append(inst)
        blk.instructions[:] = keep
    return removed


def _build_program(nchunk=NCHUNK, nchains=2):
    from contextlib import ExitStack

    import concourse.bacc as bacc
    import concourse.mybir as mybir
    import concourse.tile as tile

    f32 = mybir.dt.float32
    bf16 = mybir.dt.bfloat16
    AF = mybir.ActivationFunctionType

    nc = bacc.Bacc("TRN2", target_bir_lowering=False)

    em_d = nc.dram_tensor("em", [K, NSTEP * BCC], bf16, kind="ExternalInput")
    wt_d = nc.dram_tensor("wt", [K, K], f32, kind="ExternalInput")
    bias_d = nc.dram_tensor("bias0", [K, 1], f32, kind="ExternalInput")

    out_d = nc.dram_tensor("out", [K, BCC], bf16, kind="ExternalOutput")

    with tile.TileContext(nc) as tc, ExitStack() as ctx:
        const = ctx.enter_context(tc.tile_pool(name="const", bufs=1))
        em_pool = ctx.enter_context(tc.tile_pool(name="emp", bufs=3))
        x_pool = ctx.enter_context(tc.tile_pool(name="xp", bufs=3))
        p_pool = ctx.enter_context(tc.tile_pool(name="pp", bufs=3))
        spsum = [
            ctx.enter_context(tc.tile_pool(name=f"sp{c}", bufs=2, space="PSUM"))
            for c in range(nchains)
        ]

        # ---------------- constants ----------------
        wt_sb = const.tile([K, K], f32, tag="wt")
        nc.sync.dma_start(wt_sb[:], wt_d[:])
        W_sb = const.tile([K, K], bf16, tag="W")
        nc.scalar.activation(W_sb[:], wt_sb[:], AF.Exp)

        bias_sb = const.tile([K, 1], f32, tag="bias0")
        nc.sync.dma_start(bias_sb[:], bias_d[:])
        negg_sb = const.tile([K, 1], f32, tag="negg")
        nc.vector.memset(negg_sb[:], -G)

        # ---------------- recurrence: state = x_s * (W^T @ state) ----------
        # graduated chunk sizes: the DP can start after a small first DMA +
        # exp instead of waiting for a full 32-position chunk
        chunks = [4, 8, 8, 12] + [TCHUNK] * ((NSTEP - 32) // TCHUNK)
        assert sum(chunks) == NSTEP
        NCH = nchains
        cw = [BCC // NCH + (1 if c < BCC % NCH else 0) for c in range(NCH)]
        coff = [sum(cw[:c]) for c in range(NCH)]
        P = [None] * NCH
        pos = 0
        for ci, n in enumerate(chunks):
            if n < TCHUNK:  # ramp chunk: one-off tiles
                em_t = const.tile([K, n * BCC], bf16, tag=f"em_r{ci}")
                x_t = const.tile([K, n * BCC], bf16, tag=f"x_r{ci}")
            else:
                em_t = em_pool.tile([K, TCHUNK * BCC], bf16, tag="em")
                x_t = x_pool.tile([K, TCHUNK * BCC], bf16, tag="x")
            nc.sync.dma_start(
                em_t[:, : n * BCC], em_d[:, pos * BCC : (pos + n) * BCC]
            )
            if pos == 0:
                # state_0 = exp(em_pos0 + bias); init before the x-exp so the
                # first matmul isn't gated on the whole chunk's exp
                for c in range(NCH):
                    P[c] = p_pool.tile([K, cw[c]], bf16, tag=f"P{c}", name=f"P{c}")
                    nc.scalar.activation(
                        P[c][:], em_t[:, coff[c] : coff[c] + cw[c]], AF.Exp,
                        bias=bias_sb[:, 0:1],
                    )
            nc.scalar.activation(
                x_t[:, : n * BCC], em_t[:, : n * BCC], AF.Exp, bias=negg_sb[:]
            )

            for tl in range(n):
                s = pos + tl
                if s == 0:
                    continue

                for c in range(NCH):
                    x_sl = x_t[:, tl * BCC + coff[c] : tl * BCC + coff[c] + cw[c]]
                    S = spsum[c].tile([K, cw[c]], f32, tag=f"S{c}", name=f"S{c}")
                    nc.tensor.matmul(S[:], lhsT=W_sb[:], rhs=P[c][:],
                                     start=True, stop=True)
                    Pn = p_pool.tile([K, cw[c]], bf16, tag=f"P{c}", name=f"Pn{c}")
                    nc.vector.tensor_mul(Pn[:], S[:], x_sl)
                    P[c] = Pn
            pos += n

        # ---------------- write the final state tile ----------------
        for c in range(NCH):
            nc.sync.dma_start(out_d[:, coff[c] : coff[c] + cw[c]], P[c][:])

    nc.compile()
    _dedupe_ldweights(nc)
    return nc


def _prep_core_em(emt, bf16):
    """emt: [256, K, 128] float32 stream for one core -> [K, 256*128]."""
    return np.ascontiguousarray(
        emt.transpose(1, 0, 2).reshape(K, NSTEP * BCC)
    ).astype(bf16)


def kernel(emissions, tags, mask, start_transitions, end_transitions, transitions,
           trace=False):
    global _PROGRAM
    import concourse.mybir as mybir
    from concourse.bass_utils import run_bass_kernel_spmd

    bf16 = mybir.dt.np(mybir.dt.bfloat16)

    mask_np = np.asarray(mask)
    assert mask_np.all(), "kernel assumes an all-ones mask"

    emissions = np.asarray(emissions, dtype=np.float32)
    tg = np.asarray(tags).astype(np.int64)
    start = np.asarray(start_transitions, dtype=np.float32)
    end = np.asarray(end_transitions, dtype=np.float32)
    trans = np.asarray(transitions, dtype=np.float32)

    # ---- numerator (path score) on host, float64 ----
    emit = np.take_along_axis(emissions, tg[:, :, None], axis=2)[..., 0]
    score_total = (
        start.astype(np.float64)[tg[:, 0]].sum()
        + emit.astype(np.float64).sum()
        + trans.astype(np.float64)[tg[:, :-1], tg[:, 1:]].sum()
        + end.astype(np.float64)[tg[:, -1]].sum()
    )

    # ---- device inputs: 4 forward cores (t=0..255) + 4 backward cores ----
    emt = emissions.transpose(1, 2, 0)  # [T, K, B]
    in_maps = []
    for c in range(4):  # forward
        sub = emt[0:NSTEP, :, c * BCC : (c + 1) * BCC]
        in_maps.append({
            "em": _prep_core_em(sub, bf16),
            "wt": trans,
            "bias0": start.reshape(K, 1),
        })
    transT = np.ascontiguousarray(trans.T)
    for c in range(4):  # backward: stream positions s=0..255 are t=511..256
        sub = emt[T - 1 : T - 1 - NSTEP : -1, :, c * BCC : (c + 1) * BCC]
        in_maps.append({
            "em": _prep_core_em(np.ascontiguousarray(sub), bf16),
            "wt": transT,
            "bias0": end.reshape(K, 1),
        })

    if _PROGRAM is None:
        _PROGRAM = _build_program()

    res = run_bass_kernel_spmd(
        _PROGRAM, in_maps, core_ids=list(range(NCORES)), trace=trace
    )

    # ---- host join: Z_b = sum_{j,k} P[j,b] E[j,k] R'[k,b] ----
    E64 = np.exp(trans.astype(np.float64))
    denom_total = np.float64(0.0)
    for c in range(4):
        Pf = np.asarray(res.results[c]["out"], dtype=np.float64)       # [K, 128]
        Rb = np.asarray(res.results[4 + c]["out"], dtype=np.float64)   # [K, 128]
        Z = ((E64.T @ Pf) * Rb).sum(axis=0)                            # [128]
        denom_total += (np.log(Z) + 510.0 * G).sum()
    kernel.last_results = res
    return np.float32(score_total - denom_total)


# revision 26
# speedup vs baseline: 1.0072x; 1.0072x over previous
"""CRF loss (log-likelihood sum) on 8 Trainium2 NeuronCores.

Shapes (hardcoded): emissions (512, 512, 128) f32, tags (512, 512) i64,
mask (512, 512) bool (assumed all ones), start/end (128,) f32,
transitions (128, 128) f32.  Output: scalar f32 = sum_b llh_b.

Strategy:
  Numerator (path score) is pure index arithmetic over the inputs and is
  computed on the host in float64.

  Denominator (forward algorithm) in probability space:
      P_t = (E^T @ P_{t-1}) * exp(em_t - g),  E = exp(trans)
  i.e. the per-step logsumexp becomes a TensorE matmul (E stationary)
  followed by one elementwise multiply reading PSUM.  g is a constant
  per-step normalizer chosen so the state stays in bf16 range (validated
  offline); no renormalization needed.

  The recurrence is latency-bound (sequential in t), so the chain is cut
  in half: the identity Z_b = sum_{j,k} P_255[j,b] E[j,k] R'_256[k,b]
  splits the work into a forward recurrence over t=0..255 and an
  independent backward recurrence R'_tau = x_tau * (E @ R'_tau+1) over
  tau=511..256.  Both have the same dataflow (state = x * (W^T @ state)),
  differing only in data: W = exp(trans) vs exp(trans^T), initial bias =
  start vs end, and the order of the emission stream.  Cores 0-3 run
  forward for 128 batch columns each; cores 4-7 run backward for the same
  columns.  Each core runs one SPMD program for 256 sequential steps
  (half of the 511-step chain), and the host joins the two 128x128 final
  state tiles per column block in float64.
"""

import numpy as np

B, T, K = 512, 512, 128
NCORES = 8
BCC = 128                 # batch columns per core (4 fwd + 4 bwd cores)
TCHUNK = 32
NCHUNK = 8                # 8 chunks x 32 = 256 stream positions per core
NSTEP = NCHUNK * TCHUNK   # 256
G = 5.35                  # per-step growth normalizer (exp stays in range)

_PROGRAM = None


def _dedupe_ldweights(nc):
    """Remove redundant weight reloads: every DP-step matmul uses the same
    stationary weights, and the per-matmul LDWEIGHTS sits on the PE queue.
    The tile scheduler splits each matmul into a standalone InstLdweights
    plus a non-self-loading InstMatmult (ldweights=False), so dropping an
    InstLdweights whose weights are already resident is safe."""
    import concourse.mybir as mybir

    def sig(ap):
        try:
            if ap.regs_read():
                return None  # register-offset APs are not statically stable
            return (ap.memref, str(ap.ap), int(ap.offset), str(ap.dtype))
        except Exception:
            return None

    removed = 0
    for blk in nc.main_func.blocks:
        loaded = None
        keep = []
        for inst in blk.instructions:
            if isinstance(inst, mybir.InstLdweights):
                si = inst.sync_info
                clean = si is None or (not si.on_wait and not si.on_update)
                s = sig(inst.ins[0]) if len(inst.ins) == 1 else None
                if s is not None and s == loaded:
                    removed += 1
                    if not clean:
                        # preserve the load's sync as a PE event-sem wait
                        ev = mybir.InstEventSemaphore(
                            name=nc.get_next_instruction_name(), ins=[], outs=[]
                        )
                        ev.engine = mybir.EngineType.PE
                        ev.sync_info = inst.sync_info
                        nc.register_instruction(ev)
                        keep.append(ev)
                    continue  # weights already resident: drop the reload
                loaded = s
            elif (
                isinstance(inst, mybir.InstMatmult)
                and getattr(inst, "is_transpose", False)
            ):
                loaded = None  # transposes stream through the PE array
            keep.append(inst)
        blk.instructions[:] = keep
    return removed


def _build_program(nchunk=NCHUNK, nchains=2):
    from contextlib import ExitStack

    import concourse.bacc as bacc
    import concourse.mybir as mybir
    import concourse.tile as tile

    f32 = mybir.dt.float32
    bf16 = mybir.dt.bfloat16
    AF = mybir.ActivationFunctionType

    nc = bacc.Bacc("TRN2", target_bir_lowering=False)

    em_d = nc.dram_tensor("em", [K, NSTEP * BCC], bf16, kind="ExternalInput")
    wt_d = nc.dram_tensor("wt", [K, K], f32, kind="ExternalInput")
    bias_d = nc.dram_tensor("bias0", [K, 1], f32, kind="ExternalInput")

    out_d = nc.dram_tensor("out", [K, BCC], bf16, kind="ExternalOutput")

    with tile.TileContext(nc) as tc, ExitStack() as ctx:
        const = ctx.enter_context(tc.tile_pool(name="const", bufs=1))
        em_pool = ctx.enter_context(tc.tile_pool(name="emp", bufs=3))
        x_pool = ctx.enter_context(tc.tile_pool(name="xp", bufs=3))
        p_pool = ctx.enter_context(tc.tile_pool(name="pp", bufs=3))
        spsum = [
            ctx.enter_context(tc.tile_pool(name=f"sp{c}", bufs=2, space="PSUM"))
            for c in range(nchains)
        ]
        gpsum = ctx.enter_context(tc.tile_pool(name="gpsum", bufs=2, space="PSUM"))

        # ---------------- constants ----------------
        wt_sb = const.tile([K, K], f32, tag="wt")
        nc.sync.dma_start(wt_sb[:], wt_d[:])
        W_sb = const.tile([K, K], bf16, tag="W")
        nc.scalar.activation(W_sb[:], wt_sb[:], AF.Exp)

        bias_sb = const.tile([K, 1], f32, tag="bias0")
        nc.sync.dma_start(bias_sb[:], bias_d[:])
        negg_sb = const.tile([K, 1], f32, tag="negg")
        nc.vector.memset(negg_sb[:], -G)

        # ---------------- recurrence: state = x_s * (W^T @ state) ----------
        # graduated chunk sizes: the DP can start after a small first DMA +
        # exp instead of waiting for a full 32-position chunk
        chunks = [4, 8, 8, 12] + [TCHUNK] * ((NSTEP - 32) // TCHUNK)
        assert sum(chunks) == NSTEP
        NCH = nchains
        cw = [BCC // NCH + (1 if c < BCC % NCH else 0) for c in range(NCH)]
        coff = [sum(cw[:c]) for c in range(NCH)]
        P = [None] * NCH
        pos = 0
        for ci, n in enumerate(chunks):
            if n < TCHUNK:  # ramp chunk: one-off tiles
                em_t = const.tile([K, n * BCC], bf16, tag=f"em_r{ci}")
                x_t = const.tile([K, n * BCC], bf16, tag=f"x_r{ci}")
            else:
                em_t = em_pool.tile([K, TCHUNK * BCC], bf16, tag="em")
                x_t = x_pool.tile([K, TCHUNK * BCC], bf16, tag="x")
            nc.sync.dma_start(
                em_t[:, : n * BCC], em_d[:, pos * BCC : (pos + n) * BCC]
            )
            if pos == 0:
                # state_0 = exp(em_pos0 + bias); init before the x-exp so the
                # first matmul isn't gated on the whole chunk's exp
                for c in range(NCH):
                    P[c] = p_pool.tile([K, cw[c]], bf16, tag=f"P{c}", name=f"P{c}")
                    nc.scalar.activation(
                        P[c][:], em_t[:, coff[c] : coff[c] + cw[c]], AF.Exp,
                        bias=bias_sb[:, 0:1],
                    )
            nc.scalar.activation(
                x_t[:, : n * BCC], em_t[:, : n * BCC], AF.Exp, bias=negg_sb[:]
            )

            for tl in range(n):
                s = pos + tl
                if s == 0:
                    continue

                prevP0 = P[0]
                for c in range(NCH):
                    x_sl = x_t[:, tl * BCC + coff[c] : tl * BCC + coff[c] + cw[c]]
                    S = spsum[c].tile([K, cw[c]], f32, tag=f"S{c}", name=f"S{c}")
                    nc.tensor.matmul(S[:], lhsT=W_sb[:], rhs=P[c][:],
                                     start=True, stop=True)
                    Pn = p_pool.tile([K, cw[c]], bf16, tag=f"P{c}", name=f"Pn{c}")
                    nc.vector.tensor_mul(Pn[:], S[:], x_sl)
                    P[c] = Pn
                # ghost matmul: keeps the PE array saturated so its DVFS
                # p-state ramps (2x clock on stream+drain); pinned to this
                # iteration by reading the previous step's P tile
                gh = gpsum.tile([K, cw[0]], f32, tag="gh", name="gh")
                nc.tensor.matmul(gh[:], lhsT=W_sb[:], rhs=prevP0[:],
                                 start=True, stop=True)
            pos += n

        # ---------------- write the final state tile ----------------
        for c in range(NCH):
            nc.sync.dma_start(out_d[:, coff[c] : coff[c] + cw[c]], P[c][:])

    nc.compile()
    _dedupe_ldweights(nc)
    return nc


def _prep_core_em(emt, bf16):
    """emt: [256, K, 128] float32 stream for one core -> [K, 256*128]."""
    return np.ascontiguousarray(
        emt.transpose(1, 0, 2).reshape(K, NSTEP * BCC)
    ).astype(bf16)


def kernel(emissions, tags, mask, start_transitions, end_transitions, transitions,
           trace=False):
    global _PROGRAM
    import concourse.mybir as mybir
    from concourse.bass_utils import run_bass_kernel_spmd

    bf16 = mybir.dt.np(mybir.dt.bfloat16)

    mask_np = np.asarray(mask)
    assert mask_np.all(), "kernel assumes an all-ones mask"

    emissions = np.asarray(emissions, dtype=np.float32)
    tg = np.asarray(tags).astype(np.int64)
    start = np.asarray(start_transitions, dtype=np.float32)
    end = np.asarray(end_transitions, dtype=np.float32)
    trans = np.asarray(transitions, dtype=np.float32)

    # ---- numerator (path score) on host, float64 ----
    emit = np.take_along_axis(emissions, tg[:, :, None], axis=2)[..., 0]
    score_total = (
        start.astype(np.float64)[tg[:, 0]].sum()
        + emit.astype(np.float64).sum()
        + trans.astype(np.float64)[tg[:, :-1], tg[:, 1:]].sum()
        + end.astype(np.float64)[tg[:, -1]].sum()
    )

    # ---- device inputs: 4 forward cores (t=0..255) + 4 backward cores ----
    emt = emissions.transpose(1, 2, 0)  # [T, K, B]
    in_maps = []
    for c in range(4):  # forward
        sub = emt[0:NSTEP, :, c * BCC : (c + 1) * BCC]
        in_maps.append({
            "em": _prep_core_em(sub, bf16),
            "wt": trans,
            "bias0": start.reshape(K, 1),
        })
    transT = np.ascontiguousarray(trans.T)
    for c in range(4):  # backward: stream positions s=0..255 are t=511..256
        sub = emt[T - 1 : T - 1 - NSTEP : -1, :, c * BCC : (c + 1) * BCC]
        in_maps.append({
            "em": _prep_core_em(np.ascontiguousarray(sub), bf16),
            "wt": transT,
            "bias0": end.reshape(K, 1),
        })

    if _PROGRAM is None:
        _PROGRAM = _build_program()

    res = run_bass_kernel_spmd(
        _PROGRAM, in_maps, core_ids=list(range(NCORES)), trace=trace
    )

    # ---- host join: Z_b = sum_{j,k} P[j,b] E[j,k] R'[k,b] ----
    E64 = np.exp(trans.astype(np.float64))
    denom_total = np.float64(0.0)
    for c in range(4):
        Pf = np.asarray(res.results[c]["out"], dtype=np.float64)       # [K, 128]
        Rb = np.asarray(res.results[4 + c]["out"], dtype=np.float64)   # [K, 128]
        Z = ((E64.T @ Pf) * Rb).sum(axis=0)                            # [128]
        denom_total += (np.log(Z) + 510.0 * G).sum()
    kernel.last_results = res
    return np.float32(score_total - denom_total)


# revision 27
# speedup vs baseline: 1.0082x; 1.0009x over previous
"""CRF loss (log-likelihood sum) on 8 Trainium2 NeuronCores.

Shapes (hardcoded): emissions (512, 512, 128) f32, tags (512, 512) i64,
mask (512, 512) bool (assumed all ones), start/end (128,) f32,
transitions (128, 128) f32.  Output: scalar f32 = sum_b llh_b.

Strategy:
  Numerator (path score) is pure index arithmetic over the inputs and is
  computed on the host in float64.

  Denominator (forward algorithm) in probability space:
      P_t = (E^T @ P_{t-1}) * exp(em_t - g),  E = exp(trans)
  i.e. the per-step logsumexp becomes a TensorE matmul (E stationary)
  followed by one elementwise multiply reading PSUM.  g is a constant
  per-step normalizer chosen so the state stays in bf16 range (validated
  offline); no renormalization needed.

  The recurrence is latency-bound (sequential in t), so the chain is cut
  in half: the identity Z_b = sum_{j,k} P_255[j,b] E[j,k] R'_256[k,b]
  splits the work into a forward recurrence over t=0..255 and an
  independent backward recurrence R'_tau = x_tau * (E @ R'_tau+1) over
  tau=511..256.  Both have the same dataflow (state = x * (W^T @ state)),
  differing only in data: W = exp(trans) vs exp(trans^T), initial bias =
  start vs end, and the order of the emission stream.  Cores 0-3 run
  forward for 128 batch columns each; cores 4-7 run backward for the same
  columns.  Each core runs one SPMD program for 256 sequential steps
  (half of the 511-step chain), and the host joins the two 128x128 final
  state tiles per column block in float64.
"""

import numpy as np

B, T, K = 512, 512, 128
NCORES = 8
BCC = 128                 # batch columns per core (4 fwd + 4 bwd cores)
TCHUNK = 32
NCHUNK = 8                # 8 chunks x 32 = 256 stream positions per core
NSTEP = NCHUNK * TCHUNK   # 256
G = 5.35                  # per-step growth normalizer (exp stays in range)

_PROGRAM = None


def _dedupe_ldweights(nc):
    """Remove redundant weight reloads: every DP-step matmul uses the same
    stationary weights, and the per-matmul LDWEIGHTS sits on the PE queue.
    The tile scheduler splits each matmul into a standalone InstLdweights
    plus a non-self-loading InstMatmult (ldweights=False), so dropping an
    InstLdweights whose weights are already resident is safe."""
    import concourse.mybir as mybir

    def sig(ap):
        try:
            if ap.regs_read():
                return None  # register-offset APs are not statically stable
            return (ap.memref, str(ap.ap), int(ap.offset), str(ap.dtype))
        except Exception:
            return None

    removed = 0
    for blk in nc.main_func.blocks:
        loaded = None
        keep = []
        for inst in blk.instructions:
            if isinstance(inst, mybir.InstLdweights):
                si = inst.sync_info
                clean = si is None or (not si.on_wait and not si.on_update)
                s = sig(inst.ins[0]) if len(inst.ins) == 1 else None
                if s is not None and s == loaded:
                    removed += 1
                    if not clean:
                        # preserve the load's sync as a PE event-sem wait
                        ev = mybir.InstEventSemaphore(
                            name=nc.get_next_instruction_name(), ins=[], outs=[]
                        )
                        ev.engine = mybir.EngineType.PE
                        ev.sync_info = inst.sync_info
                        nc.register_instruction(ev)
                        keep.append(ev)
                    continue  # weights already resident: drop the reload
                loaded = s
            elif (
                isinstance(inst, mybir.InstMatmult)
                and getattr(inst, "is_transpose", False)
            ):
                loaded = None  # transposes stream through the PE array
            keep.append(inst)
        blk.instructions[:] = keep
    return removed


def _build_program(nchunk=NCHUNK, nchains=2):
    from contextlib import ExitStack

    import concourse.bacc as bacc
    import concourse.mybir as mybir
    import concourse.tile as tile

    f32 = mybir.dt.float32
    bf16 = mybir.dt.bfloat16
    AF = mybir.ActivationFunctionType

    nc = bacc.Bacc("TRN2", target_bir_lowering=False)

    em_d = nc.dram_tensor("em", [K, NSTEP * BCC], bf16, kind="ExternalInput")
    wt_d = nc.dram_tensor("wt", [K, K], f32, kind="ExternalInput")
    bias_d = nc.dram_tensor("bias0", [K, 1], f32, kind="ExternalInput")

    out_d = nc.dram_tensor("out", [K, BCC], bf16, kind="ExternalOutput")

    with tile.TileContext(nc) as tc, ExitStack() as ctx:
        const = ctx.enter_context(tc.tile_pool(name="const", bufs=1))
        em_pool = ctx.enter_context(tc.tile_pool(name="emp", bufs=3))
        x_pool = ctx.enter_context(tc.tile_pool(name="xp", bufs=3))
        p_pool = ctx.enter_context(tc.tile_pool(name="pp", bufs=3))
        spsum = [
            ctx.enter_context(tc.tile_pool(name=f"sp{c}", bufs=2, space="PSUM"))
            for c in range(nchains)
        ]
        gpsum = ctx.enter_context(tc.tile_pool(name="gpsum", bufs=2, space="PSUM"))

        # ---------------- constants ----------------
        wt_sb = const.tile([K, K], f32, tag="wt")
        nc.sync.dma_start(wt_sb[:], wt_d[:])
        W_sb = const.tile([K, K], bf16, tag="W")
        nc.scalar.activation(W_sb[:], wt_sb[:], AF.Exp)

        bias_sb = const.tile([K, 1], f32, tag="bias0")
        nc.sync.dma_start(bias_sb[:], bias_d[:])
        negg_sb = const.tile([K, 1], f32, tag="negg")
        nc.vector.memset(negg_sb[:], -G)

        # ---------------- recurrence: state = x_s * (W^T @ state) ----------
        # graduated chunk sizes: the DP can start after a small first DMA +
        # exp instead of waiting for a full 32-position chunk
        chunks = [4, 8, 8, 12] + [TCHUNK] * ((NSTEP - 32) // TCHUNK)
        assert sum(chunks) == NSTEP
        NCH = nchains
        cw = [BCC // NCH + (1 if c < BCC % NCH else 0) for c in range(NCH)]
        coff = [sum(cw[:c]) for c in range(NCH)]
        P = [None] * NCH
        pos = 0
        for ci, n in enumerate(chunks):
            if n < TCHUNK:  # ramp chunk: one-off tiles
                em_t = const.tile([K, n * BCC], bf16, tag=f"em_r{ci}")
                x_t = const.tile([K, n * BCC], bf16, tag=f"x_r{ci}")
            else:
                em_t = em_pool.tile([K, TCHUNK * BCC], bf16, tag="em")
                x_t = x_pool.tile([K, TCHUNK * BCC], bf16, tag="x")
            nc.sync.dma_start(
                em_t[:, : n * BCC], em_d[:, pos * BCC : (pos + n) * BCC]
            )
            if pos == 0:
                # state_0 = exp(em_pos0 + bias); init before the x-exp so the
                # first matmul isn't gated on the whole chunk's exp
                for c in range(NCH):
                    P[c] = p_pool.tile([K, cw[c]], bf16, tag=f"P{c}", name=f"P{c}")
                    nc.scalar.activation(
                        P[c][:], em_t[:, coff[c] : coff[c] + cw[c]], AF.Exp,
                        bias=bias_sb[:, 0:1],
                    )
            nc.scalar.activation(
                x_t[:, : n * BCC], em_t[:, : n * BCC], AF.Exp, bias=negg_sb[:]
            )

            for tl in range(n):
                s = pos + tl
                if s == 0:
                    continue

                prevP0 = P[0]
                for c in range(NCH):
                    x_sl = x_t[:, tl * BCC + coff[c] : tl * BCC + coff[c] + cw[c]]
                    S = spsum[c].tile([K, cw[c]], f32, tag=f"S{c}", name=f"S{c}")
                    nc.tensor.matmul(S[:], lhsT=W_sb[:], rhs=P[c][:],
                                     start=True, stop=True)
                    Pn = p_pool.tile([K, cw[c]], bf16, tag=f"P{c}", name=f"Pn{c}")
                    nc.vector.tensor_mul(Pn[:], S[:], x_sl)
                    P[c] = Pn
                # ghost matmul: keeps the PE streaming through the multiply
                # window so the next real matmul issues back-to-back; pinned
                # to this iteration by reading the previous step's P tile
                gh = gpsum.tile([K, cw[0]], f32, tag="gh", name="gh")
                nc.tensor.matmul(gh[:], lhsT=W_sb[:], rhs=prevP0[:],
                                 start=True, stop=True)
                # DVE ghosts: keep the vector engine's access pipe warm
                # across its idle window so the critical multiply starts
                # back-to-back (warm ~139ns vs cold ~224ns)
                for gi, src in enumerate((prevP0, P[1])):
                    gv = p_pool.tile([K, cw[0]], bf16, tag="gv", name=f"gv{gi}")
                    nc.vector.tensor_scalar_mul(gv[:], src[:], 1.0)
            pos += n

        # ---------------- write the final state tile ----------------
        for c in range(NCH):
            nc.sync.dma_start(out_d[:, coff[c] : coff[c] + cw[c]], P[c][:])

    nc.compile()
    _dedupe_ldweights(nc)
    return nc


def _prep_core_em(emt, bf16):
    """emt: [256, K, 128] float32 stream for one core -> [K, 256*128]."""
    return np.ascontiguousarray(
        emt.transpose(1, 0, 2).reshape(K, NSTEP * BCC)
    ).astype(bf16)


def kernel(emissions, tags, mask, start_transitions, end_transitions, transitions,
           trace=False):
    global _PROGRAM
    import concourse.mybir as mybir
    from concourse.bass_utils import run_bass_kernel_spmd

    bf16 = mybir.dt.np(mybir.dt.bfloat16)

    mask_np = np.asarray(mask)
    assert mask_np.all(), "kernel assumes an all-ones mask"

    emissions = np.asarray(emissions, dtype=np.float32)
    tg = np.asarray(tags).astype(np.int64)
    start = np.asarray(start_transitions, dtype=np.float32)
    end = np.asarray(end_transitions, dtype=np.float32)
    trans = np.asarray(transitions, dtype=np.float32)

    # ---- numerator (path score) on host, float64 ----
    emit = np.take_along_axis(emissions, tg[:, :, None], axis=2)[..., 0]
    score_total = (
        start.astype(np.float64)[tg[:, 0]].sum()
        + emit.astype(np.float64).sum()
        + trans.astype(np.float64)[tg[:, :-1], tg[:, 1:]].sum()
        + end.astype(np.float64)[tg[:, -1]].sum()
    )

    # ---- device inputs: 4 forward cores (t=0..255) + 4 backward cores ----
    emt = emissions.transpose(1, 2, 0)  # [T, K, B]
    in_maps = []
    for c in range(4):  # forward
        sub = emt[0:NSTEP, :, c * BCC : (c + 1) * BCC]
        in_maps.append({
            "em": _prep_core_em(sub, bf16),
            "wt": trans,
            "bias0": start.reshape(K, 1),
        })
    transT = np.ascontiguousarray(trans.T)
    for c in range(4):  # backward: stream positions s=0..255 are t=511..256
        sub = emt[T - 1 : T - 1 - NSTEP : -1, :, c * BCC : (c + 1) * BCC]
        in_maps.append({
            "em": _prep_core_em(np.ascontiguousarray(sub), bf16),
            "wt": transT,
            "bias0": end.reshape(K, 1),
        })

    if _PROGRAM is None:
        _PROGRAM = _build_program()

    res = run_bass_kernel_spmd(
        _PROGRAM, in_maps, core_ids=list(range(NCORES)), trace=trace
    )

    # ---- host join: Z_b = sum_{j,k} P[j,b] E[j,k] R'[k,b] ----
    E64 = np.exp(trans.astype(np.float64))
    denom_total = np.float64(0.0)
    for c in range(4):
        Pf = np.asarray(res.results[c]["out"], dtype=np.float64)       # [K, 128]
        Rb = np.asarray(res.results[4 + c]["out"], dtype=np.float64)   # [K, 128]
        Z = ((E64.T @ Pf) * Rb).sum(axis=0)                            # [128]
        denom_total += (np.log(Z) + 510.0 * G).sum()
    kernel.last_results = res
    return np.float32(score_total - denom_total)


# revision 30
# speedup vs baseline: 1.0354x; 1.0270x over previous
"""CRF loss (log-likelihood sum) on 8 Trainium2 NeuronCores.

Shapes (hardcoded): emissions (512, 512, 128) f32, tags (512, 512) i64,
mask (512, 512) bool (assumed all ones), start/end (128,) f32,
transitions (128, 128) f32.  Output: scalar f32 = sum_b llh_b.

Strategy:
  Numerator (path score) is pure index arithmetic over the inputs and is
  computed on the host in float64.

  Denominator (forward algorithm) in probability space:
      P_t = (E^T @ P_{t-1}) * exp(em_t - g),  E = exp(trans)
  i.e. the per-step logsumexp becomes a TensorE matmul (E stationary)
  followed by one elementwise multiply reading PSUM.  g is a constant
  per-step normalizer chosen so the state stays in bf16 range (validated
  offline); no renormalization needed.

  The recurrence is latency-bound (sequential in t), so the chain is cut
  in half: the identity Z_b = sum_{j,k} P_255[j,b] E[j,k] R'_256[k,b]
  splits the work into a forward recurrence over t=0..255 and an
  independent backward recurrence R'_tau = x_tau * (E @ R'_tau+1) over
  tau=511..256.  Both have the same dataflow (state = x * (W^T @ state)),
  differing only in data: W = exp(trans) vs exp(trans^T), initial bias =
  start vs end, and the order of the emission stream.  Cores 0-3 run
  forward for 128 batch columns each; cores 4-7 run backward for the same
  columns.  Each core runs one SPMD program for 256 sequential steps
  (half of the 511-step chain), and the host joins the two 128x128 final
  state tiles per column block in float64.
"""

import numpy as np

B, T, K = 512, 512, 128
NCORES = 8
BCC = 128                 # batch columns per core (4 fwd + 4 bwd cores)
TCHUNK = 32
NCHUNK = 8                # 8 chunks x 32 = 256 stream positions per core
NSTEP = NCHUNK * TCHUNK   # 256
G = 5.35                  # per-step growth normalizer (exp stays in range)

_PROGRAM = None


def _dedupe_ldweights(nc):
    """Remove redundant weight reloads: every DP-step matmul uses the same
    stationary weights, and the per-matmul LDWEIGHTS sits on the PE queue.
    The tile scheduler splits each matmul into a standalone InstLdweights
    plus a non-self-loading InstMatmult (ldweights=False), so dropping an
    InstLdweights whose weights are already resident is safe."""
    import concourse.mybir as mybir

    def sig(ap):
        try:
            if ap.regs_read():
                return None  # register-offset APs are not statically stable
            return (ap.memref, str(ap.ap), int(ap.offset), str(ap.dtype))
        except Exception:
            return None

    removed = 0
    for blk in nc.main_func.blocks:
        loaded = None
        keep = []
        for inst in blk.instructions:
            if isinstance(inst, mybir.InstLdweights):
                si = inst.sync_info
                clean = si is None or (not si.on_wait and not si.on_update)
                s = sig(inst.ins[0]) if len(inst.ins) == 1 else None
                if s is not None and s == loaded:
                    removed += 1
                    if not clean:
                        # preserve the load's sync as a PE event-sem wait
                        ev = mybir.InstEventSemaphore(
                            name=nc.get_next_instruction_name(), ins=[], outs=[]
                        )
                        ev.engine = mybir.EngineType.PE
                        ev.sync_info = inst.sync_info
                        nc.register_instruction(ev)
                        keep.append(ev)
                    continue  # weights already resident: drop the reload
                loaded = s
            elif (
                isinstance(inst, mybir.InstMatmult)
                and getattr(inst, "is_transpose", False)
            ):
                loaded = None  # transposes stream through the PE array
            keep.append(inst)
        blk.instructions[:] = keep
    return removed


def _build_program(nchunk=NCHUNK, nchains=2):
    from contextlib import ExitStack

    import concourse.bacc as bacc
    import concourse.mybir as mybir
    import concourse.tile as tile

    f32 = mybir.dt.float32
    bf16 = mybir.dt.bfloat16
    AF = mybir.ActivationFunctionType

    nc = bacc.Bacc("TRN2", target_bir_lowering=False)

    em_d = nc.dram_tensor("em", [K, NSTEP * BCC], bf16, kind="ExternalInput")
    wt_d = nc.dram_tensor("wt", [K, K], f32, kind="ExternalInput")
    bias_d = nc.dram_tensor("bias0", [K, 1], f32, kind="ExternalInput")

    out_d = nc.dram_tensor("out", [K, BCC], bf16, kind="ExternalOutput")

    with tile.TileContext(nc) as tc, ExitStack() as ctx:
        const = ctx.enter_context(tc.tile_pool(name="const", bufs=1))
        em_pool = ctx.enter_context(tc.tile_pool(name="emp", bufs=3))
        x_pool = ctx.enter_context(tc.tile_pool(name="xp", bufs=3))
        p_pool = ctx.enter_context(tc.tile_pool(name="pp", bufs=3))
        spsum = [
            ctx.enter_context(tc.tile_pool(name=f"sp{c}", bufs=2, space="PSUM"))
            for c in range(nchains)
        ]
        gpsum = ctx.enter_context(tc.tile_pool(name="gpsum", bufs=2, space="PSUM"))

        # ---------------- constants ----------------
        wt_sb = const.tile([K, K], f32, tag="wt")
        nc.sync.dma_start(wt_sb[:], wt_d[:])
        W_sb = const.tile([K, K], bf16, tag="W")
        nc.scalar.activation(W_sb[:], wt_sb[:], AF.Exp)

        bias_sb = const.tile([K, 1], f32, tag="bias0")
        nc.sync.dma_start(bias_sb[:], bias_d[:])
        negg_sb = const.tile([K, 1], f32, tag="negg")
        nc.vector.memset(negg_sb[:], -G)

        # ---------------- recurrence: state = x_s * (W^T @ state) ----------
        # graduated chunk sizes: the DP can start after a small first DMA +
        # exp instead of waiting for a full 32-position chunk
        chunks = [4, 8, 8, 12] + [TCHUNK] * ((NSTEP - 32) // TCHUNK)
        assert sum(chunks) == NSTEP
        NCH = nchains
        cw = [BCC // NCH + (1 if c < BCC % NCH else 0) for c in range(NCH)]
        coff = [sum(cw[:c]) for c in range(NCH)]
        P = [None] * NCH
        pos = 0
        for ci, n in enumerate(chunks):
            if n < TCHUNK:  # ramp chunk: one-off tiles
                em_t = const.tile([K, n * BCC], bf16, tag=f"em_r{ci}")
                x_t = const.tile([K, n * BCC], bf16, tag=f"x_r{ci}")
            else:
                em_t = em_pool.tile([K, TCHUNK * BCC], bf16, tag="em")
                x_t = x_pool.tile([K, TCHUNK * BCC], bf16, tag="x")
            nc.sync.dma_start(
                em_t[:, : n * BCC], em_d[:, pos * BCC : (pos + n) * BCC]
            )
            if pos == 0:
                # state_0 = exp(em_pos0 + bias); init before the x-exp so the
                # first matmul isn't gated on the whole chunk's exp
                for c in range(NCH):
                    P[c] = p_pool.tile([K, cw[c]], bf16, tag=f"P{c}", name=f"P{c}")
                    nc.scalar.activation(
                        P[c][:], em_t[:, coff[c] : coff[c] + cw[c]], AF.Exp,
                        bias=bias_sb[:, 0:1],
                    )
            nc.scalar.activation(
                x_t[:, : n * BCC], em_t[:, : n * BCC], AF.Exp, bias=negg_sb[:]
            )

            for tl in range(n):
                s = pos + tl
                if s == 0:
                    continue

                prevP0, prevP1 = P[0], P[1]
                for c in range(NCH):
                    x_sl = x_t[:, tl * BCC + coff[c] : tl * BCC + coff[c] + cw[c]]
                    S = spsum[c].tile([K, cw[c]], f32, tag=f"S{c}", name=f"S{c}")
                    nc.tensor.matmul(S[:], lhsT=W_sb[:], rhs=P[c][:],
                                     start=True, stop=True)
                    Pn = p_pool.tile([K, cw[c]], bf16, tag=f"P{c}", name=f"Pn{c}")
                    nc.vector.tensor_mul(Pn[:], S[:], x_sl)
                    P[c] = Pn
                # ghost matmuls: keep the PE array streaming continuously so
                # its DVFS p-state ramps to the high clock (halves stream and
                # drain time on the critical matmuls).  Each reads the
                # previous step's P tile through a stride-0-repeated AP (512
                # columns), which both pins it to this iteration and makes it
                # wide enough to cover the PE's idle window.
                for gi, src in enumerate((prevP0, prevP1)):
                    import bass_rust as _br
                    rep = src[:].copy()
                    rep.ap = _br.VecI64Pair(
                        [list(rep.ap[0]), [0, 512 // cw[gi]], list(rep.ap[-1])]
                    )
                    gh = gpsum.tile([K, 512], f32, tag="gh", name=f"gh{gi}")
                    nc.tensor.matmul(gh[:], lhsT=W_sb[:], rhs=rep,
                                     start=True, stop=True)
            pos += n

        # ---------------- write the final state tile ----------------
        for c in range(NCH):
            nc.sync.dma_start(out_d[:, coff[c] : coff[c] + cw[c]], P[c][:])

    nc.compile()
    _dedupe_ldweights(nc)
    return nc


def _prep_core_em(emt, bf16):
    """emt: [256, K, 128] float32 stream for one core -> [K, 256*128]."""
    return np.ascontiguousarray(
        emt.transpose(1, 0, 2).reshape(K, NSTEP * BCC)
    ).astype(bf16)


def kernel(emissions, tags, mask, start_transitions, end_transitions, transitions,
           trace=False):
    global _PROGRAM
    import concourse.mybir as mybir
    from concourse.bass_utils import run_bass_kernel_spmd

    bf16 = mybir.dt.np(mybir.dt.bfloat16)

    mask_np = np.asarray(mask)
    assert mask_np.all(), "kernel assumes an all-ones mask"

    emissions = np.asarray(emissions, dtype=np.float32)
    tg = np.asarray(tags).astype(np.int64)
    start = np.asarray(start_transitions, dtype=np.float32)
    end = np.asarray(end_transitions, dtype=np.float32)
    trans = np.asarray(transitions, dtype=np.float32)

    # ---- numerator (path score) on host, float64 ----
    emit = np.take_along_axis(emissions, tg[:, :, None], axis=2)[..., 0]
    score_total = (
        start.astype(np.float64)[tg[:, 0]].sum()
        + emit.astype(np.float64).sum()
        + trans.astype(np.float64)[tg[:, :-1], tg[:, 1:]].sum()
        + end.astype(np.float64)[tg[:, -1]].sum()
    )

    # ---- device inputs: 4 forward cores (t=0..255) + 4 backward cores ----
    emt = emissions.transpose(1, 2, 0)  # [T, K, B]
    in_maps = []
    for c in range(4):  # forward
        sub = emt[0:NSTEP, :, c * BCC : (c + 1) * BCC]
        in_maps.append({
            "em": _prep_core_em(sub, bf16),
            "wt": trans,
            "bias0": start.reshape(K, 1),
        })
    transT = np.ascontiguousarray(trans.T)
    for c in range(4):  # backward: stream positions s=0..255 are t=511..256
        sub = emt[T - 1 : T - 1 - NSTEP : -1, :, c * BCC : (c + 1) * BCC]
        in_maps.append({
            "em": _prep_core_em(np.ascontiguousarray(sub), bf16),
            "wt": transT,
            "bias0": end.reshape(K, 1),
        })

    if _PROGRAM is None:
        _PROGRAM = _build_program()

    res = run_bass_kernel_spmd(
        _PROGRAM, in_maps, core_ids=list(range(NCORES)), trace=trace
    )

    # ---- host join: Z_b = sum_{j,k} P[j,b] E[j,k] R'[k,b] ----
    E64 = np.exp(trans.astype(np.float64))
    denom_total = np.float64(0.0)
    for c in range(4):
        Pf = np.asarray(res.results[c]["out"], dtype=np.float64)       # [K, 128]
        Rb = np.asarray(res.results[4 + c]["out"], dtype=np.float64)   # [K, 128]
        Z = ((E64.T @ Pf) * Rb).sum(axis=0)                            # [128]
        denom_total += (np.log(Z) + 510.0 * G).sum()
    kernel.last_results = res
    return np.float32(score_total - denom_total)
